# revision 7
# baseline (speedup 1.0000x reference)
"""AdaConv2D (instance-norm -> grouped 3x3 conv -> grouped 1x1 conv -> bias) on 8 TRN2 cores.

Strategy (pure data parallel, 1 sample per core):
  - Host: fuse pw o dw into one effective grouped 3x3 conv (group size 4), pack the
    per-group 4x4 blocks into block-diagonal 32x32 lhsT tiles (8 groups per tile).
  - Device, per 128-channel chunk: contiguous DMA into a row-padded SBUF layout
    (one zero row above/below, rows of 128 contiguous), bn_stats/bn_aggr for the
    instance-norm statistics, one in-place ACT pass to normalize, then the conv as
    16 concurrent 32x32 TensorE tiles (4 channel sub-chunks x 4 spatial tiles) with
    9 shifted-AP taps accumulating in PSUM.  W-edge padding is handled by shrinking
    the free dim of the dw=+-1 taps (those output columns simply don't receive the
    tap).  Bias is folded into the PSUM eviction.
"""
import os
import sys
import numpy as np

if "/opt/trn_rl_repo" not in sys.path:
    sys.path.insert(0, "/opt/trn_rl_repo")

B, C, H, W = 8, 512, 128, 128
HW = H * W            # 16384
NCH = 4               # 128-channel chunks per sample
NTAP = 9
ROWS_PAD = H + 2      # 130 rows of 128 in padded SBUF layout
PADF = ROWS_PAD * W   # 16640 elems per partition
EPS = 1e-7
# taps ordered so the first three are dw=0 (full-width writes -> correct PSUM init)
TAPS = [(0, 1), (1, 1), (2, 1), (0, 0), (1, 0), (2, 0), (0, 2), (1, 2), (2, 2)]

_CACHE = {}


def _build_program():
    import concourse.bass as bass
    import concourse.tile as tile
    from concourse import bacc, mybir

    f32 = mybir.dt.float32
    nc = bacc.Bacc("TRN2", target_bir_lowering=False, debug=False,
                   enable_asserts=False, num_devices=8)

    x_d = nc.dram_tensor("x", [C, HW], f32, kind="ExternalInput")
    w_d = nc.dram_tensor("w", [128, NCH * NTAP * 32], f32, kind="ExternalInput")
    b_d = nc.dram_tensor("bias", [128, 16], f32, kind="ExternalInput")
    out_d = nc.dram_tensor("out", [C, HW], f32, kind="ExternalOutput")

    # out viewed as [cc, t, p, r, e] for the per-spatial-tile store
    out_v = out_d[:].rearrange("(a r p) (t e) -> a t p r e", a=NCH, r=4, p=32,
                               t=32, e=512)

    with tile.TileContext(nc) as tc:
        with (
            tc.tile_pool(name="xpool", bufs=2) as xpool,
            tc.tile_pool(name="wpool", bufs=1) as wpool,
            tc.tile_pool(name="spool", bufs=8) as spool,
            tc.tile_pool(name="opool", bufs=3) as opool,
            tc.tile_pool(name="psum", bufs=8, space=bass.MemorySpace.PSUM) as pspool,
        ):
            w_sb = wpool.tile([128, NCH * NTAP * 32], f32)
            nc.sync.dma_start(w_sb[:], w_d[:])
            bias_sb = wpool.tile([128, 16], f32)
            nc.sync.dma_start(bias_sb[:], b_d[:])

            for cc in range(NCH):
                xt = xpool.tile([128, PADF], f32, tag="xt")
                # zero halo rows (top row 0, bottom row 129)
                nc.gpsimd.memset(xt[:, 0:W], 0.0)
                nc.gpsimd.memset(xt[:, PADF - W:PADF], 0.0)
                # load interior in 4 slices so stats can start early
                stats6 = spool.tile([128, 32 * 6], f32, tag="stats")
                for k in range(4):
                    nc.sync.dma_start(xt[:, W + k * 4096: W + (k + 1) * 4096],
                                      x_d[cc * 128:(cc + 1) * 128,
                                          k * 4096:(k + 1) * 4096])
                # bn_stats: 512 elems max per call -> 8 calls per DMA slice
                for j in range(32):
                    nc.vector.bn_stats(stats6[:, j * 6:(j + 1) * 6],
                                       xt[:, W + j * 512: W + (j + 1) * 512])
                mv = spool.tile([128, 2], f32, tag="mv")
                nc.vector.bn_aggr(mv[:], stats6[:].rearrange("p (h s) -> p h s", s=6))
                stdv = spool.tile([128, 1], f32, tag="stdv")
                # std_unbiased = sqrt(var * N/(N-1)); then += eps ; rstd = 1/(...)
                nc.scalar.activation(stdv[:], mv[:, 1:2],
                                     mybir.ActivationFunctionType.Sqrt,
                                     scale=float(HW) / float(HW - 1))
                stde = spool.tile([128, 1], f32, tag="stde")
                nc.vector.tensor_scalar_add(stde[:], stdv[:], EPS)
                rstd = spool.tile([128, 1], f32, tag="rstd")
                nc.vector.reciprocal(rstd[:], stde[:])
                nmean = spool.tile([128, 1], f32, tag="nmean")
                nc.vector.tensor_scalar_mul(nmean[:], mv[:, 0:1], -1.0)
                nmr = spool.tile([128, 1], f32, tag="nmr")
                nc.vector.tensor_mul(nmr[:], nmean[:], rstd[:])
                # normalize in place: xt = xt * rstd + (-mean*rstd)
                nc.scalar.activation(xt[:, W:W + HW], xt[:, W:W + HW],
                                     mybir.ActivationFunctionType.Identity,
                                     bias=nmr[:, 0:1], scale=rstd[:, 0:1])

                for q in range(8):
                    pb = [pspool.tile([128, 512], f32, tag="pb", name=f"pb{cc}_{q}_{r}")
                          for r in range(4)]
                    for ti, (dh, dwi) in enumerate(TAPS):
                        start, stop = (ti == 0), (ti == NTAP - 1)
                        tapi = dh * 3 + dwi
                        for r in range(4):
                            lhsT = w_sb[32 * r:32 * r + 32,
                                        (cc * NTAP + tapi) * 32:
                                        (cc * NTAP + tapi) * 32 + 32]
                            for c in range(4):
                                t = 4 * q + c
                                base = (4 * t + dh) * W
                                outp = pb[r][32 * c:32 * c + 32, :]
                                tp = (32 * r, 32 * c)
                                if dwi == 1:
                                    nc.tensor.matmul(
                                        outp, lhsT,
                                        xt[32 * r:32 * r + 32, base:base + 512],
                                        start=start, stop=stop, tile_position=tp)
                                else:
                                    o3 = outp.rearrange("p (h w) -> p h w", w=W)
                                    r3 = xt[32 * r:32 * r + 32,
                                            base:base + 512].rearrange(
                                                "p (h w) -> p h w", w=W)
                                    if dwi == 0:   # dw=-1
                                        nc.tensor.matmul(
                                            o3[:, :, 1:W], lhsT, r3[:, :, 0:W - 1],
                                            start=start, stop=stop,
                                            skip_group_check=True, tile_position=tp)
                                    else:          # dw=+1
                                        nc.tensor.matmul(
                                            o3[:, :, 0:W - 1], lhsT, r3[:, :, 1:W],
                                            start=start, stop=stop,
                                            skip_group_check=True, tile_position=tp)
                    out_sb = opool.tile([128, 2048], f32, tag="osb")
                    for r in range(4):
                        bias_ap = bias_sb[:, cc * 4 + r: cc * 4 + r + 1]
                        dst = out_sb[:, r * 512:(r + 1) * 512]
                        if r % 2 == 0:
                            nc.vector.tensor_scalar_add(dst, pb[r][:, :], bias_ap)
                        else:
                            nc.scalar.activation(
                                dst, pb[r][:, :],
                                mybir.ActivationFunctionType.Identity,
                                bias=bias_ap, scale=1.0)
                    for c in range(4):
                        nc.scalar.dma_start(
                            out_v[cc, 4 * q + c],
                            out_sb[32 * c:32 * c + 32, :].rearrange(
                                "p (r e) -> p r e", e=512))
    nc.compile()
    return nc


def _pack_inputs(x, dw, pw, biases):
    """Host-side: fuse pw o dw, scatter into block-diag 32x32 lhsT tiles."""
    G = 128
    dwr = dw.reshape(B, G, 4, 4, 3, 3)          # [b, g, m, i, kh, kw]
    pwr = pw.reshape(B, G, 4, 4)                # [b, g, j, m]
    eff = np.einsum('bgjm,bgmikl->bgjikl', pwr, dwr)  # [b, g, j, i, kh, kw]
    # w_host[b, 32r + 4gl + i, (cc*9+tap)*32 + 4gl + j] = eff[b, g, j, i, dh, dwi]
    # vectorized scatter:
    w_host = np.zeros((B, 128, NCH * NTAP * 32), dtype=np.float32)
    wv = w_host.reshape(B, 4, 8, 4, NCH, NTAP, 8, 4)  # [b, r, gl_k, i, cc, tap, gl_m, j]
    er = eff.reshape(B, NCH, 4, 8, 4, 4, NTAP)        # [b, cc, r, gl, j, i, tap]
    for gl in range(8):
        # eff[b, cc*32 + r*8 + gl, j, i, tap] -> wv[b, r, gl, i, cc, tap, gl, j]
        e = er[:, :, :, gl]                     # [b, cc, r, j, i, tap]
        wv[:, :, gl, :, :, :, gl, :] = e.transpose(0, 2, 4, 1, 5, 3)
    bias_host = np.zeros((B, 128, 16), dtype=np.float32)
    bfull = biases.reshape(B, C)
    p = np.arange(128)
    for cc in range(NCH):
        for r in range(4):
            bias_host[:, :, cc * 4 + r] = bfull[:, cc * 128 + 32 * r + (p % 32)]
    return w_host, bias_host


def kernel(x, dw_kernels, pw_kernels, biases):
    from concourse.bass_utils import run_bass_kernel_spmd

    x = np.ascontiguousarray(np.asarray(x, dtype=np.float32))
    dw = np.asarray(dw_kernels, dtype=np.float32)
    pw = np.asarray(pw_kernels, dtype=np.float32)
    bs = np.asarray(biases, dtype=np.float32)

    if "nc" not in _CACHE:
        _CACHE["nc"] = _build_program()
    nc = _CACHE["nc"]

    w_host, bias_host = _pack_inputs(x, dw, pw, bs)
    in_maps = [{"x": x[i].reshape(C, HW),
                "w": w_host[i],
                "bias": bias_host[i]} for i in range(B)]
    res = run_bass_kernel_spmd(nc, in_maps, core_ids=list(range(B)),
                               trace=bool(int(os.environ.get("KTRACE", "0"))))
    _CACHE["last_result"] = res
    out = np.stack([res.results[i]["out"].reshape(C, H, W) for i in range(B)])
    return out


# revision 8
# speedup vs baseline: 1.0388x; 1.0388x over previous
"""AdaConv2D (instance-norm -> grouped 3x3 conv -> grouped 1x1 conv -> bias) on 8 TRN2 cores.

V2 strategy (pure data parallel, 1 sample per core):
  - Host: fuse pw o dw into one effective grouped 3x3 conv (group size 4), pack the
    per-group 4x4 blocks into block-diagonal 32x32 bf16 lhsT tiles (8 groups/tile).
  - Device, per 128-channel chunk:
      * SWDGE DMA casts x f32->bf16 into a row-padded SBUF layout (one zero row
        above/below, 128-elem rows contiguous).
      * bn_stats/bn_aggr -> mean/rstd; one in-place ACT pass normalizes.
      * conv: 16 concurrent 32x32 TensorE tiles = 4 channel sub-chunks (row groups)
        x 4 spatial quarters (col groups); 9 shifted-AP taps accumulate in PSUM
        (bf16 single-pass matmuls).  W-edges handled by shrinking the free dim of
        dw=+-1 taps.  Spatial tile of col group c at step q is t = 8c + q, so each
        partition group owns a contiguous spatial quarter.
      * eviction (bias folded) stages 4 steps of output in SBUF, then one
        3-dim DMA per (quarter, half) stores 8KB-contiguous runs.
"""
import os
import sys
import numpy as np
import ml_dtypes

if "/opt/trn_rl_repo" not in sys.path:
    sys.path.insert(0, "/opt/trn_rl_repo")

B, C, H, W = 8, 512, 128, 128
HW = H * W            # 16384
NCH = 4               # 128-channel chunks per sample
NTAP = 9
ROWS_PAD = H + 2      # 130 rows of 128 in padded SBUF layout
PADF = ROWS_PAD * W   # 16640 elems per partition
EPS = 1e-7
# taps ordered so the first three are dw=0 (full-width writes -> correct PSUM init)
TAPS = [(0, 1), (1, 1), (2, 1), (0, 0), (1, 0), (2, 0), (0, 2), (1, 2), (2, 2)]

_CACHE = {}


def _build_program():
    import concourse.bass as bass
    import concourse.tile as tile
    from concourse import bacc, mybir

    f32 = mybir.dt.float32
    bf16 = mybir.dt.bfloat16
    nc = bacc.Bacc("TRN2", target_bir_lowering=False, debug=False,
                   enable_asserts=False, num_devices=8)

    x_d = nc.dram_tensor("x", [C, HW], f32, kind="ExternalInput")
    w_d = nc.dram_tensor("w", [128, NCH * NTAP * 32], bf16, kind="ExternalInput")
    b_d = nc.dram_tensor("bias", [128, 16], f32, kind="ExternalInput")
    out_d = nc.dram_tensor("out", [C, HW], f32, kind="ExternalOutput")

    # store view: [cc, c(quarter), h(half), p, r, e(2048)]
    out_v = out_d[:].rearrange("(a r p) (c h e) -> a c h p r e", a=NCH, r=4, p=32,
                               c=4, h=2, e=2048)

    with tile.TileContext(nc) as tc:
        with (
            tc.tile_pool(name="xpool", bufs=2) as xpool,
            tc.tile_pool(name="wpool", bufs=1) as wpool,
            tc.tile_pool(name="spool", bufs=8) as spool,
            tc.tile_pool(name="opool", bufs=2) as opool,
            tc.tile_pool(name="psum", bufs=8, space=bass.MemorySpace.PSUM) as pspool,
        ):
            w_sb = wpool.tile([128, NCH * NTAP * 32], bf16)
            nc.sync.dma_start(w_sb[:], w_d[:])
            bias_sb = wpool.tile([128, 16], f32)
            nc.sync.dma_start(bias_sb[:], b_d[:])

            for cc in range(NCH):
                xt = xpool.tile([128, PADF], bf16, tag="xt")
                # zero halo rows (top row 0, bottom row 129)
                nc.gpsimd.memset(xt[:, 0:W], 0.0)
                nc.gpsimd.memset(xt[:, PADF - W:PADF], 0.0)
                # load interior in 4 slices (SWDGE f32->bf16 cast) so stats start early
                stats6 = spool.tile([128, 32 * 6], f32, tag="stats")
                for k in range(4):
                    nc.gpsimd.dma_start(xt[:, W + k * 4096: W + (k + 1) * 4096],
                                        x_d[cc * 128:(cc + 1) * 128,
                                            k * 4096:(k + 1) * 4096])
                for j in range(32):
                    nc.vector.bn_stats(stats6[:, j * 6:(j + 1) * 6],
                                       xt[:, W + j * 512: W + (j + 1) * 512])
                mv = spool.tile([128, 2], f32, tag="mv")
                nc.vector.bn_aggr(mv[:], stats6[:].rearrange("p (h s) -> p h s", s=6))
                stdv = spool.tile([128, 1], f32, tag="stdv")
                nc.scalar.activation(stdv[:], mv[:, 1:2],
                                     mybir.ActivationFunctionType.Sqrt,
                                     scale=float(HW) / float(HW - 1))
                stde = spool.tile([128, 1], f32, tag="stde")
                nc.vector.tensor_scalar_add(stde[:], stdv[:], EPS)
                rstd = spool.tile([128, 1], f32, tag="rstd")
                nc.vector.reciprocal(rstd[:], stde[:])
                nmean = spool.tile([128, 1], f32, tag="nmean")
                nc.vector.tensor_scalar_mul(nmean[:], mv[:, 0:1], -1.0)
                nmr = spool.tile([128, 1], f32, tag="nmr")
                nc.vector.tensor_mul(nmr[:], nmean[:], rstd[:])
                # normalize in place: xt = xt * rstd + (-mean*rstd)
                nc.scalar.activation(xt[:, W:W + HW], xt[:, W:W + HW],
                                     mybir.ActivationFunctionType.Identity,
                                     bias=nmr[:, 0:1], scale=rstd[:, 0:1])

                for half in range(2):
                    om = opool.tile([128, 4 * 2048], f32, tag="om")
                    for qq in range(4):
                        q = half * 4 + qq
                        pb = [pspool.tile([128, 512], f32, tag="pb",
                                          name=f"pb{cc}_{q}_{r}") for r in range(4)]
                        for ti, (dh, dwi) in enumerate(TAPS):
                            start, stop = (ti == 0), (ti == NTAP - 1)
                            tapi = dh * 3 + dwi
                            for r in range(4):
                                lhsT = w_sb[32 * r:32 * r + 32,
                                            (cc * NTAP + tapi) * 32:
                                            (cc * NTAP + tapi) * 32 + 32]
                                for c in range(4):
                                    t = 8 * c + q
                                    base = (4 * t + dh) * W
                                    outp = pb[r][32 * c:32 * c + 32, :]
                                    tp = (32 * r, 32 * c)
                                    if dwi == 1:
                                        nc.tensor.matmul(
                                            outp, lhsT,
                                            xt[32 * r:32 * r + 32, base:base + 512],
                                            start=start, stop=stop, tile_position=tp)
                                    else:
                                        o3 = outp.rearrange("p (h w) -> p h w", w=W)
                                        r3 = xt[32 * r:32 * r + 32,
                                                base:base + 512].rearrange(
                                                    "p (h w) -> p h w", w=W)
                                        if dwi == 0:   # dw=-1
                                            nc.tensor.matmul(
                                                o3[:, :, 1:W], lhsT, r3[:, :, 0:W - 1],
                                                start=start, stop=stop,
                                                skip_group_check=True, tile_position=tp)
                                        else:          # dw=+1
                                            nc.tensor.matmul(
                                                o3[:, :, 0:W - 1], lhsT, r3[:, :, 1:W],
                                                start=start, stop=stop,
                                                skip_group_check=True, tile_position=tp)
                        for r in range(4):
                            bias_ap = bias_sb[:, cc * 4 + r: cc * 4 + r + 1]
                            dst = om[:, r * 2048 + qq * 512: r * 2048 + qq * 512 + 512]
                            if r % 2 == 0:
                                nc.vector.tensor_scalar_add(dst, pb[r][:, :], bias_ap)
                            else:
                                nc.scalar.activation(
                                    dst, pb[r][:, :],
                                    mybir.ActivationFunctionType.Identity,
                                    bias=bias_ap, scale=1.0)
                    for c in range(4):
                        nc.sync.dma_start(
                            out_v[cc, c, half],
                            om[32 * c:32 * c + 32, :].rearrange(
                                "p (r e) -> p r e", e=2048))
    nc.compile()
    return nc


def _pack_inputs(x, dw, pw, biases):
    """Host-side: fuse pw o dw, scatter into block-diag 32x32 lhsT tiles."""
    G = 128
    dwr = dw.reshape(B, G, 4, 4, 3, 3)          # [b, g, m, i, kh, kw]
    pwr = pw.reshape(B, G, 4, 4)                # [b, g, j, m]
    eff = np.einsum('bgjm,bgmikl->bgjikl', pwr, dwr)  # [b, g, j, i, kh, kw]
    # w_host[b, 32r + 4gl + i, (cc*9+tap)*32 + 4gl + j] = eff[b, g, j, i, dh, dwi]
    w_host = np.zeros((B, 128, NCH * NTAP * 32), dtype=np.float32)
    wv = w_host.reshape(B, 4, 8, 4, NCH, NTAP, 8, 4)  # [b, r, gl_k, i, cc, tap, gl_m, j]
    er = eff.reshape(B, NCH, 4, 8, 4, 4, NTAP)        # [b, cc, r, gl, j, i, tap]
    for gl in range(8):
        e = er[:, :, :, gl]                     # [b, cc, r, j, i, tap]
        wv[:, :, gl, :, :, :, gl, :] = e.transpose(0, 2, 4, 1, 5, 3)
    bias_host = np.zeros((B, 128, 16), dtype=np.float32)
    bfull = biases.reshape(B, C)
    p = np.arange(128)
    for cc in range(NCH):
        for r in range(4):
            bias_host[:, :, cc * 4 + r] = bfull[:, cc * 128 + 32 * r + (p % 32)]
    return w_host.astype(ml_dtypes.bfloat16), bias_host


def kernel(x, dw_kernels, pw_kernels, biases):
    from concourse.bass_utils import run_bass_kernel_spmd

    x = np.ascontiguousarray(np.asarray(x, dtype=np.float32))
    dw = np.asarray(dw_kernels, dtype=np.float32)
    pw = np.asarray(pw_kernels, dtype=np.float32)
    bs = np.asarray(biases, dtype=np.float32)

    if "nc" not in _CACHE:
        _CACHE["nc"] = _build_program()
    nc = _CACHE["nc"]

    w_host, bias_host = _pack_inputs(x, dw, pw, bs)
    in_maps = [{"x": x[i].reshape(C, HW),
                "w": w_host[i],
                "bias": bias_host[i]} for i in range(B)]
    res = run_bass_kernel_spmd(nc, in_maps, core_ids=list(range(B)),
                               trace=bool(int(os.environ.get("KTRACE", "0"))))
    _CACHE["last_result"] = res
    out = np.stack([res.results[i]["out"].reshape(C, H, W) for i in range(B)])
    return out


# revision 10
# speedup vs baseline: 1.4098x; 1.3571x over previous
"""AdaConv2D (instance-norm -> grouped 3x3 conv -> grouped 1x1 conv -> bias) on 8 TRN2 cores.

V3 strategy (pure data parallel, 1 sample per core):
  - Host: fuse pw o dw into one effective grouped 3x3 conv (group size 4), pack the
    per-group 4x4 blocks into block-diagonal 32x32 bf16 lhsT tiles (8 groups/tile).
  - Device, per 128-channel chunk:
      * SWDGE DMA casts x f32->bf16 into a row-padded SBUF layout (zero halo rows).
      * bn_stats/bn_aggr -> mean/rstd.
      * normalize runs in 8 row-band PIECES (round-robin DVE/ACT/GpSimd) so the conv
        quad q only depends on pieces <= q+1 -- normalization stays off the
        critical path and the TensorE pipeline never drains between chunks.
      * conv: 16 concurrent 32x32 TensorE tiles = 4 channel sub-chunks (row groups)
        x 4 spatial quarters (col groups); 9 shifted-AP taps accumulate in PSUM.
        Spatial tile of col group c at step q is t = 8c + q.
      * eviction (bias folded, bf16 out) fills a full-chunk staging tile; one
        3-dim DMA per quarter stores 8KB-contiguous runs.  Output DRAM is bf16;
        the host upcasts to f32 (rel-err budget 2e-2 >> bf16 rounding).
"""
import os
import sys
import numpy as np
import ml_dtypes

if "/opt/trn_rl_repo" not in sys.path:
    sys.path.insert(0, "/opt/trn_rl_repo")

B, C, H, W = 8, 512, 128, 128
HW = H * W            # 16384
NCH = 4               # 128-channel chunks per sample
NTAP = 9
ROWS_PAD = H + 2      # 130 rows of 128 in padded SBUF layout
PADF = ROWS_PAD * W   # 16640 elems per partition
EPS = 1e-7
# taps ordered so the first three are dw=0 (full-width writes -> correct PSUM init)
TAPS = [(0, 1), (1, 1), (2, 1), (0, 0), (1, 0), (2, 0), (0, 2), (1, 2), (2, 2)]

_CACHE = {}


def _build_program():
    import concourse.bass as bass
    import concourse.tile as tile
    from concourse import bacc, mybir

    f32 = mybir.dt.float32
    bf16 = mybir.dt.bfloat16
    MULT = mybir.AluOpType.mult
    ADD = mybir.AluOpType.add
    nc = bacc.Bacc("TRN2", target_bir_lowering=False, debug=False,
                   enable_asserts=False, num_devices=8)

    x_d = nc.dram_tensor("x", [C, HW], f32, kind="ExternalInput")
    w_d = nc.dram_tensor("w", [128, NCH * NTAP * 32], bf16, kind="ExternalInput")
    b_d = nc.dram_tensor("bias", [128, 16], f32, kind="ExternalInput")
    out_d = nc.dram_tensor("out", [C, HW], bf16, kind="ExternalOutput")

    # store view: [cc, c(quarter), p, r, e(4096)]
    out_v = out_d[:].rearrange("(a r p) (c e) -> a c p r e", a=NCH, r=4, p=32,
                               c=4, e=4096)

    with tile.TileContext(nc) as tc:
        with (
            tc.tile_pool(name="xpool", bufs=2) as xpool,
            tc.tile_pool(name="wpool", bufs=1) as wpool,
            tc.tile_pool(name="spool", bufs=8) as spool,
            tc.tile_pool(name="opool", bufs=2) as opool,
            tc.tile_pool(name="psum", bufs=8, space=bass.MemorySpace.PSUM) as pspool,
        ):
            w_sb = wpool.tile([128, NCH * NTAP * 32], bf16)
            nc.sync.dma_start(w_sb[:], w_d[:])
            bias_sb = wpool.tile([128, 16], f32)
            nc.sync.dma_start(bias_sb[:], b_d[:])

            for cc in range(NCH):
                xt = xpool.tile([128, PADF], bf16, tag="xt")
                # zero halo rows (top row 0, bottom row 129)
                nc.gpsimd.memset(xt[:, 0:W], 0.0)
                nc.gpsimd.memset(xt[:, PADF - W:PADF], 0.0)
                # load interior in 4 slices (SWDGE f32->bf16 cast) so stats start early
                stats6 = spool.tile([128, 32 * 6], f32, tag="stats")
                for k in range(4):
                    nc.gpsimd.dma_start(xt[:, W + k * 4096: W + (k + 1) * 4096],
                                        x_d[cc * 128:(cc + 1) * 128,
                                            k * 4096:(k + 1) * 4096])
                for j in range(32):
                    nc.vector.bn_stats(stats6[:, j * 6:(j + 1) * 6],
                                       xt[:, W + j * 512: W + (j + 1) * 512])
                mv = spool.tile([128, 2], f32, tag="mv")
                nc.vector.bn_aggr(mv[:], stats6[:].rearrange("p (h s) -> p h s", s=6))
                stdv = spool.tile([128, 1], f32, tag="stdv")
                nc.scalar.activation(stdv[:], mv[:, 1:2],
                                     mybir.ActivationFunctionType.Sqrt,
                                     scale=float(HW) / float(HW - 1))
                stde = spool.tile([128, 1], f32, tag="stde")
                nc.vector.tensor_scalar_add(stde[:], stdv[:], EPS)
                rstd = spool.tile([128, 1], f32, tag="rstd")
                nc.vector.reciprocal(rstd[:], stde[:])
                nmr = spool.tile([128, 1], f32, tag="nmr")
                # nmr = (mean * -1) * rstd in one DVE op
                nc.vector.scalar_tensor_tensor(nmr[:], mv[:, 0:1], -1.0, rstd[:],
                                               op0=MULT, op1=MULT)
                rstd_ap = rstd[:, 0:1]
                nmr_ap = nmr[:, 0:1]

                # normalize in 8 row-band pieces: piece q covers rows [4q, 4q+4)
                # of every 32-row quarter band (conv quad q depends on pieces <= q+1)
                full = xt[:, W:W + HW].rearrange("p (b r w) -> p b r w", b=4, r=32)
                for q in range(8):
                    eng = [nc.vector, nc.scalar, nc.gpsimd][q % 3]
                    tgt = full[:, :, 4 * q:4 * q + 4, :]
                    if eng is nc.scalar:
                        nc.scalar.activation(tgt, tgt,
                                             mybir.ActivationFunctionType.Identity,
                                             bias=nmr_ap, scale=rstd_ap)
                    else:
                        eng.tensor_scalar(tgt, tgt, rstd_ap, nmr_ap,
                                          op0=MULT, op1=ADD)

                om = opool.tile([128, 4 * 4096], bf16, tag="om")
                for q in range(8):
                    pb = [pspool.tile([128, 512], f32, tag="pb",
                                      name=f"pb{cc}_{q}_{r}") for r in range(4)]
                    for ti, (dh, dwi) in enumerate(TAPS):
                        start, stop = (ti == 0), (ti == NTAP - 1)
                        tapi = dh * 3 + dwi
                        for r in range(4):
                            lhsT = w_sb[32 * r:32 * r + 32,
                                        (cc * NTAP + tapi) * 32:
                                        (cc * NTAP + tapi) * 32 + 32]
                            for c in range(4):
                                t = 8 * c + q
                                base = (4 * t + dh) * W
                                outp = pb[r][32 * c:32 * c + 32, :]
                                tp = (32 * r, 32 * c)
                                if dwi == 1:
                                    nc.tensor.matmul(
                                        outp, lhsT,
                                        xt[32 * r:32 * r + 32, base:base + 512],
                                        start=start, stop=stop, tile_position=tp)
                                else:
                                    o3 = outp.rearrange("p (h w) -> p h w", w=W)
                                    r3 = xt[32 * r:32 * r + 32,
                                            base:base + 512].rearrange(
                                                "p (h w) -> p h w", w=W)
                                    if dwi == 0:   # dw=-1
                                        nc.tensor.matmul(
                                            o3[:, :, 1:W], lhsT, r3[:, :, 0:W - 1],
                                            start=start, stop=stop,
                                            skip_group_check=True, tile_position=tp)
                                    else:          # dw=+1
                                        nc.tensor.matmul(
                                            o3[:, :, 0:W - 1], lhsT, r3[:, :, 1:W],
                                            start=start, stop=stop,
                                            skip_group_check=True, tile_position=tp)
                    for r in range(4):
                        bias_ap = bias_sb[:, cc * 4 + r: cc * 4 + r + 1]
                        dst = om[:, r * 4096 + q * 512: r * 4096 + q * 512 + 512]
                        if r == 3:
                            nc.scalar.activation(
                                dst, pb[r][:, :],
                                mybir.ActivationFunctionType.Identity,
                                bias=bias_ap, scale=1.0)
                        else:
                            nc.vector.tensor_scalar_add(dst, pb[r][:, :], bias_ap)
                for c in range(4):
                    nc.sync.dma_start(
                        out_v[cc, c],
                        om[32 * c:32 * c + 32, :].rearrange(
                            "p (r e) -> p r e", e=4096))
    nc.compile()
    return nc


def _pack_inputs(x, dw, pw, biases):
    """Host-side: fuse pw o dw, scatter into block-diag 32x32 lhsT tiles."""
    G = 128
    dwr = dw.reshape(B, G, 4, 4, 3, 3)          # [b, g, m, i, kh, kw]
    pwr = pw.reshape(B, G, 4, 4)                # [b, g, j, m]
    eff = np.einsum('bgjm,bgmikl->bgjikl', pwr, dwr)  # [b, g, j, i, kh, kw]
    # w_host[b, 32r + 4gl + i, (cc*9+tap)*32 + 4gl + j] = eff[b, g, j, i, dh, dwi]
    w_host = np.zeros((B, 128, NCH * NTAP * 32), dtype=np.float32)
    wv = w_host.reshape(B, 4, 8, 4, NCH, NTAP, 8, 4)  # [b, r, gl_k, i, cc, tap, gl_m, j]
    er = eff.reshape(B, NCH, 4, 8, 4, 4, NTAP)        # [b, cc, r, gl, j, i, tap]
    for gl in range(8):
        e = er[:, :, :, gl]                     # [b, cc, r, j, i, tap]
        wv[:, :, gl, :, :, :, gl, :] = e.transpose(0, 2, 4, 1, 5, 3)
    bias_host = np.zeros((B, 128, 16), dtype=np.float32)
    bfull = biases.reshape(B, C)
    p = np.arange(128)
    for cc in range(NCH):
        for r in range(4):
            bias_host[:, :, cc * 4 + r] = bfull[:, cc * 128 + 32 * r + (p % 32)]
    return w_host.astype(ml_dtypes.bfloat16), bias_host


def kernel(x, dw_kernels, pw_kernels, biases):
    from concourse.bass_utils import run_bass_kernel_spmd

    x = np.ascontiguousarray(np.asarray(x, dtype=np.float32))
    dw = np.asarray(dw_kernels, dtype=np.float32)
    pw = np.asarray(pw_kernels, dtype=np.float32)
    bs = np.asarray(biases, dtype=np.float32)

    if "nc" not in _CACHE:
        _CACHE["nc"] = _build_program()
    nc = _CACHE["nc"]

    w_host, bias_host = _pack_inputs(x, dw, pw, bs)
    in_maps = [{"x": x[i].reshape(C, HW),
                "w": w_host[i],
                "bias": bias_host[i]} for i in range(B)]
    res = run_bass_kernel_spmd(nc, in_maps, core_ids=list(range(B)),
                               trace=bool(int(os.environ.get("KTRACE", "0"))))
    _CACHE["last_result"] = res
    out = np.stack([res.results[i]["out"].astype(np.float32).reshape(C, H, W)
                    for i in range(B)])
    return out


# revision 12
# speedup vs baseline: 1.4145x; 1.0034x over previous
"""AdaConv2D (instance-norm -> grouped 3x3 conv -> grouped 1x1 conv -> bias) on 8 TRN2 cores.

V4 strategy (pure data parallel, 1 sample per core):
  - Host: fuse pw o dw into one effective grouped 3x3 conv (group size 4), pack the
    per-group 4x4 blocks into block-diagonal 32x32 bf16 lhsT tiles (8 groups/tile).
  - Device, per 128-channel chunk, software-pipelined 3 deep with strict engine
    specialization (each engine runs its queue in scheduled order, so cross-chunk
    work must not share an engine queue with blocking predecessors):
      * GpSimd: halo memsets, SWDGE f32->bf16 cast DMA issues, and ALL normalize
        pieces (8 row-band pieces; conv quad q depends only on pieces <= q+1).
      * DVE: bn_stats (emitted interleaved into the PREVIOUS chunk's eviction
        stream at quad boundaries), bn_aggr, rstd chain, eviction of banks 0-1.
      * ACT: sqrt, eviction of banks 2-3.
      * TensorE: 16 concurrent 32x32 tiles (4 channel sub-chunks x 4 spatial
        quarters), 9 shifted-AP taps accumulating in PSUM; spatial tile of col
        group c at step q is t = 8c + q.  W-edges via shrunken free dims.
      * Output staged in bf16, stored per half-quarter (4KB runs); host upcasts.
"""
import os
import sys
import numpy as np
import ml_dtypes

if "/opt/trn_rl_repo" not in sys.path:
    sys.path.insert(0, "/opt/trn_rl_repo")

B, C, H, W = 8, 512, 128, 128
HW = H * W            # 16384
NCH = 4               # 128-channel chunks per sample
NTAP = 9
ROWS_PAD = H + 2      # 130 rows of 128 in padded SBUF layout
PADF = ROWS_PAD * W   # 16640 elems per partition
EPS = 1e-7
# taps ordered so the first three are dw=0 (full-width writes -> correct PSUM init)
TAPS = [(0, 1), (1, 1), (2, 1), (0, 0), (1, 0), (2, 0), (0, 2), (1, 2), (2, 2)]

_CACHE = {}


def _build_program():
    import concourse.bass as bass
    import concourse.tile as tile
    from concourse import bacc, mybir

    f32 = mybir.dt.float32
    bf16 = mybir.dt.bfloat16
    MULT = mybir.AluOpType.mult
    ADD = mybir.AluOpType.add
    IDENT = mybir.ActivationFunctionType.Identity
    nc = bacc.Bacc("TRN2", target_bir_lowering=False, debug=False,
                   enable_asserts=False, num_devices=8)

    x_d = nc.dram_tensor("x", [C, HW], f32, kind="ExternalInput")
    w_d = nc.dram_tensor("w", [128, NCH * NTAP * 32], bf16, kind="ExternalInput")
    b_d = nc.dram_tensor("bias", [128, 16], f32, kind="ExternalInput")
    out_d = nc.dram_tensor("out", [C, HW], bf16, kind="ExternalOutput")

    # store view: [cc, c(quarter), h(half), p, r, e(2048)]
    out_v = out_d[:].rearrange("(a r p) (c h e) -> a c h p r e", a=NCH, r=4, p=32,
                               c=4, h=2, e=2048)

    with tile.TileContext(nc) as tc:
        with (
            tc.tile_pool(name="xpool", bufs=3) as xpool,
            tc.tile_pool(name="wpool", bufs=1) as wpool,
            tc.tile_pool(name="spool", bufs=3) as spool,
            tc.tile_pool(name="opool", bufs=2) as opool,
            tc.tile_pool(name="psum", bufs=8, space=bass.MemorySpace.PSUM) as pspool,
        ):
            w_sb = wpool.tile([128, NCH * NTAP * 32], bf16)
            nc.sync.dma_start(w_sb[:], w_d[:])
            bias_sb = wpool.tile([128, 16], f32)
            nc.sync.dma_start(bias_sb[:], b_d[:])

            st = {}  # per-chunk small tiles

            def emit_load(cc):
                xt = xpool.tile([128, PADF], bf16, tag="xt", name=f"xt{cc}")
                st[cc] = {"xt": xt}
                nc.gpsimd.memset(xt[:, 0:W], 0.0)
                nc.gpsimd.memset(xt[:, PADF - W:PADF], 0.0)
                for k in range(4):
                    nc.gpsimd.dma_start(xt[:, W + k * 4096: W + (k + 1) * 4096],
                                        x_d[cc * 128:(cc + 1) * 128,
                                            k * 4096:(k + 1) * 4096])

            def emit_stats_block(cc, blk):
                s = st[cc]
                if blk == 0:
                    s["stats6"] = spool.tile([128, 32 * 6], f32, tag="stats",
                                             name=f"st{cc}")
                xt = s["xt"]
                for j in range(8 * blk, 8 * blk + 8):
                    nc.vector.bn_stats(s["stats6"][:, j * 6:(j + 1) * 6],
                                       xt[:, W + j * 512: W + (j + 1) * 512])

            def emit_chain(cc):
                s = st[cc]
                mv = spool.tile([128, 2], f32, tag="mv", name=f"mv{cc}")
                nc.vector.bn_aggr(mv[:], s["stats6"][:].rearrange(
                    "p (h s) -> p h s", s=6))
                stdv = spool.tile([128, 1], f32, tag="stdv", name=f"sd{cc}")
                nc.scalar.activation(stdv[:], mv[:, 1:2],
                                     mybir.ActivationFunctionType.Sqrt,
                                     scale=float(HW) / float(HW - 1))
                stde = spool.tile([128, 1], f32, tag="stde", name=f"se{cc}")
                nc.vector.tensor_scalar_add(stde[:], stdv[:], EPS)
                rstd = spool.tile([128, 1], f32, tag="rstd", name=f"rs{cc}")
                nc.vector.reciprocal(rstd[:], stde[:])
                nmr = spool.tile([128, 1], f32, tag="nmr", name=f"nm{cc}")
                nc.vector.scalar_tensor_tensor(nmr[:], mv[:, 0:1], -1.0, rstd[:],
                                               op0=MULT, op1=MULT)
                s["rstd"] = rstd
                s["nmr"] = nmr

            def emit_norm(cc, engines=(None,)):
                # 8 row-band pieces; piece q covers rows [4q,4q+4) of each band
                s = st[cc]
                full = s["xt"][:, W:W + HW].rearrange("p (b r w) -> p b r w",
                                                      b=4, r=32)
                for q in range(8):
                    eng = engines[q % len(engines)]
                    tgt = full[:, :, 4 * q:4 * q + 4, :]
                    if eng is nc.scalar:
                        nc.scalar.activation(tgt, tgt, IDENT,
                                             bias=s["nmr"][:, 0:1],
                                             scale=s["rstd"][:, 0:1])
                    else:
                        eng.tensor_scalar(tgt, tgt, s["rstd"][:, 0:1],
                                          s["nmr"][:, 0:1], op0=MULT, op1=ADD)

            def emit_quad_mms(cc, q):
                xt = st[cc]["xt"]
                pb = [pspool.tile([128, 512], f32, tag="pb",
                                  name=f"pb{cc}_{q}_{r}") for r in range(4)]
                for ti, (dh, dwi) in enumerate(TAPS):
                    start, stop = (ti == 0), (ti == NTAP - 1)
                    tapi = dh * 3 + dwi
                    for r in range(4):
                        lhsT = w_sb[32 * r:32 * r + 32,
                                    (cc * NTAP + tapi) * 32:
                                    (cc * NTAP + tapi) * 32 + 32]
                        for c in range(4):
                            t = 8 * c + q
                            base = (4 * t + dh) * W
                            outp = pb[r][32 * c:32 * c + 32, :]
                            tp = (32 * r, 32 * c)
                            if dwi == 1:
                                nc.tensor.matmul(
                                    outp, lhsT,
                                    xt[32 * r:32 * r + 32, base:base + 512],
                                    start=start, stop=stop, tile_position=tp)
                            else:
                                o3 = outp.rearrange("p (h w) -> p h w", w=W)
                                r3 = xt[32 * r:32 * r + 32,
                                        base:base + 512].rearrange(
                                            "p (h w) -> p h w", w=W)
                                if dwi == 0:   # dw=-1
                                    nc.tensor.matmul(
                                        o3[:, :, 1:W], lhsT, r3[:, :, 0:W - 1],
                                        start=start, stop=stop,
                                        skip_group_check=True, tile_position=tp)
                                else:          # dw=+1
                                    nc.tensor.matmul(
                                        o3[:, :, 0:W - 1], lhsT, r3[:, :, 1:W],
                                        start=start, stop=stop,
                                        skip_group_check=True, tile_position=tp)
                return pb

            def emit_evac(cc, q, pb, om):
                for r in range(4):
                    bias_ap = bias_sb[:, cc * 4 + r: cc * 4 + r + 1]
                    dst = om[:, r * 4096 + q * 512: r * 4096 + q * 512 + 512]
                    if r < 2:
                        nc.vector.tensor_scalar_add(dst, pb[r][:, :], bias_ap)
                    else:
                        nc.scalar.activation(dst, pb[r][:, :], IDENT,
                                             bias=bias_ap, scale=1.0)

            def emit_out(cc, om, half):
                for c in range(4):
                    nc.sync.dma_start(
                        out_v[cc, c, half],
                        om[32 * c:32 * c + 32, :].rearrange(
                            "p (r h e) -> p r h e", h=2, e=2048)[:, :, half, :])

            # ---- prologue: chunk 0 (and chunk 1 load) up front
            emit_load(0)
            emit_load(1)
            for blk in range(4):
                emit_stats_block(0, blk)
            emit_chain(0)
            emit_norm(0, engines=(nc.gpsimd, nc.vector, nc.scalar))

            for cc in range(NCH):
                om = opool.tile([128, 4 * 4096], bf16, tag="om", name=f"om{cc}")
                for q in range(8):
                    pb = emit_quad_mms(cc, q)
                    emit_evac(cc, q, pb, om)
                    if cc + 1 < NCH:
                        if q == 0 and cc + 2 < NCH:
                            emit_load(cc + 2)
                        if 2 <= q <= 5:
                            emit_stats_block(cc + 1, q - 2)
                        if q == 5:
                            emit_chain(cc + 1)
                        if q == 6:
                            emit_norm(cc + 1, engines=(nc.gpsimd,))
                    if q == 3:
                        emit_out(cc, om, 0)
                emit_out(cc, om, 1)
    nc.compile()
    return nc


def _pack_inputs(x, dw, pw, biases):
    """Host-side: fuse pw o dw, scatter into block-diag 32x32 lhsT tiles."""
    G = 128
    dwr = dw.reshape(B, G, 4, 4, 3, 3)          # [b, g, m, i, kh, kw]
    pwr = pw.reshape(B, G, 4, 4)                # [b, g, j, m]
    eff = np.einsum('bgjm,bgmikl->bgjikl', pwr, dwr)  # [b, g, j, i, kh, kw]
    w_host = np.zeros((B, 128, NCH * NTAP * 32), dtype=np.float32)
    wv = w_host.reshape(B, 4, 8, 4, NCH, NTAP, 8, 4)  # [b, r, gl_k, i, cc, tap, gl_m, j]
    er = eff.reshape(B, NCH, 4, 8, 4, 4, NTAP)        # [b, cc, r, gl, j, i, tap]
    for gl in range(8):
        e = er[:, :, :, gl]                     # [b, cc, r, j, i, tap]
        wv[:, :, gl, :, :, :, gl, :] = e.transpose(0, 2, 4, 1, 5, 3)
    bias_host = np.zeros((B, 128, 16), dtype=np.float32)
    bfull = biases.reshape(B, C)
    p = np.arange(128)
    for cc in range(NCH):
        for r in range(4):
            bias_host[:, :, cc * 4 + r] = bfull[:, cc * 128 + 32 * r + (p % 32)]
    return w_host.astype(ml_dtypes.bfloat16), bias_host


def kernel(x, dw_kernels, pw_kernels, biases):
    from concourse.bass_utils import run_bass_kernel_spmd

    x = np.ascontiguousarray(np.asarray(x, dtype=np.float32))
    dw = np.asarray(dw_kernels, dtype=np.float32)
    pw = np.asarray(pw_kernels, dtype=np.float32)
    bs = np.asarray(biases, dtype=np.float32)

    if "nc" not in _CACHE:
        _CACHE["nc"] = _build_program()
    nc = _CACHE["nc"]

    w_host, bias_host = _pack_inputs(x, dw, pw, bs)
    in_maps = [{"x": x[i].reshape(C, HW),
                "w": w_host[i],
                "bias": bias_host[i]} for i in range(B)]
    res = run_bass_kernel_spmd(nc, in_maps, core_ids=list(range(B)),
                               trace=bool(int(os.environ.get("KTRACE", "0"))))
    _CACHE["last_result"] = res
    out = np.stack([res.results[i]["out"].astype(np.float32).reshape(C, H, W)
                    for i in range(B)])
    return out


# revision 17
# speedup vs baseline: 1.4957x; 1.0574x over previous
"""AdaConv2D (instance-norm -> grouped 3x3 conv -> grouped 1x1 conv -> bias) on 8 TRN2 cores.

V4 strategy (pure data parallel, 1 sample per core):
  - Host: fuse pw o dw into one effective grouped 3x3 conv (group size 4), pack the
    per-group 4x4 blocks into block-diagonal 32x32 bf16 lhsT tiles (8 groups/tile).
  - Device, per 128-channel chunk, software-pipelined 3 deep with strict engine
    specialization (each engine runs its queue in scheduled order, so cross-chunk
    work must not share an engine queue with blocking predecessors):
      * GpSimd: halo memsets, SWDGE f32->bf16 cast DMA issues, and ALL normalize
        pieces (8 row-band pieces; conv quad q depends only on pieces <= q+1).
      * DVE: bn_stats (emitted interleaved into the PREVIOUS chunk's eviction
        stream at quad boundaries), bn_aggr, rstd chain, eviction of banks 0-1.
      * ACT: sqrt, eviction of banks 2-3.
      * TensorE: 16 concurrent 32x32 tiles (4 channel sub-chunks x 4 spatial
        quarters), 9 shifted-AP taps accumulating in PSUM; spatial tile of col
        group c at step q is t = 8c + q.  W-edges via shrunken free dims.
      * Output staged in bf16, stored per half-quarter (4KB runs); host upcasts.
"""
import os
import sys
import numpy as np
import ml_dtypes

if "/opt/trn_rl_repo" not in sys.path:
    sys.path.insert(0, "/opt/trn_rl_repo")

B, C, H, W = 8, 512, 128, 128
HW = H * W            # 16384
NCH = 4               # 128-channel chunks per sample
NTAP = 9
ROWS_PAD = H + 2      # 130 rows of 128 in padded SBUF layout
PADF = ROWS_PAD * W   # 16640 elems per partition
EPS = 1e-7
# taps ordered so the first three are dw=0 (full-width writes -> correct PSUM init)
TAPS = [(0, 1), (1, 1), (2, 1), (0, 0), (1, 0), (2, 0), (0, 2), (1, 2), (2, 2)]

_CACHE = {}


def _build_program():
    import concourse.bass as bass
    import concourse.tile as tile
    from concourse import bacc, mybir

    f32 = mybir.dt.float32
    bf16 = mybir.dt.bfloat16
    MULT = mybir.AluOpType.mult
    ADD = mybir.AluOpType.add
    IDENT = mybir.ActivationFunctionType.Identity
    nc = bacc.Bacc("TRN2", target_bir_lowering=False, debug=False,
                   enable_asserts=False, num_devices=8)

    x_d = nc.dram_tensor("x", [C, HW], f32, kind="ExternalInput")
    w_d = nc.dram_tensor("w", [128, NCH * NTAP * 32], bf16, kind="ExternalInput")
    b_d = nc.dram_tensor("bias", [128, 16], f32, kind="ExternalInput")
    out_d = nc.dram_tensor("out", [C, HW], bf16, kind="ExternalOutput")

    # store view: [cc, c(quarter), h(half), p, r, e(2048)]
    out_v = out_d[:].rearrange("(a r p) (c h e) -> a c h p r e", a=NCH, r=4, p=32,
                               c=4, h=2, e=2048)

    with tile.TileContext(nc) as tc:
        with (
            tc.tile_pool(name="xpool", bufs=3) as xpool,
            tc.tile_pool(name="wpool", bufs=1) as wpool,
            tc.tile_pool(name="spool", bufs=3) as spool,
            tc.tile_pool(name="opool", bufs=2) as opool,
            tc.tile_pool(name="psum", bufs=8, space=bass.MemorySpace.PSUM) as pspool,
        ):
            w_sb = wpool.tile([128, NCH * NTAP * 32], bf16)
            nc.sync.dma_start(w_sb[:], w_d[:])
            bias_sb = wpool.tile([128, 16], f32)
            nc.sync.dma_start(bias_sb[:], b_d[:])
            trash = wpool.tile([128, 4096], bf16)

            st = {}  # per-chunk small tiles

            def emit_load(cc):
                xt = xpool.tile([128, PADF], bf16, tag="xt", name=f"xt{cc}")
                st[cc] = {"xt": xt}
                nc.gpsimd.memset(xt[:, 0:W], 0.0)
                nc.gpsimd.memset(xt[:, PADF - W:PADF], 0.0)
                for k in range(4):
                    nc.gpsimd.dma_start(xt[:, W + k * 4096: W + (k + 1) * 4096],
                                        x_d[cc * 128:(cc + 1) * 128,
                                            k * 4096:(k + 1) * 4096])

            def emit_stats_block(cc, blk):
                # chunk-0 (prologue) path: bn_stats on DVE
                s = st[cc]
                if blk == 0:
                    s["stats6"] = spool.tile([128, 32 * 6], f32, tag="stats",
                                             name=f"st{cc}")
                xt = s["xt"]
                for j in range(8 * blk, 8 * blk + 8):
                    nc.vector.bn_stats(s["stats6"][:, j * 6:(j + 1) * 6],
                                       xt[:, W + j * 512: W + (j + 1) * 512])

            def emit_stats_act(cc, k):
                # steady-state path: ACT accumulates sum (Copy) and sumsq (Square)
                # of DMA slice k into acc cols; zero DVE cost.
                s = st[cc]
                if k == 0:
                    s["acc"] = spool.tile([128, 8], f32, tag="acc", name=f"ac{cc}")
                xt = s["xt"]
                sl = xt[:, W + k * 4096: W + (k + 1) * 4096]
                nc.scalar.activation(trash[:], sl,
                                     mybir.ActivationFunctionType.Copy,
                                     accum_out=s["acc"][:, k:k + 1])
                nc.scalar.activation(trash[:], sl,
                                     mybir.ActivationFunctionType.Square,
                                     accum_out=s["acc"][:, k + 4:k + 5])

            def emit_chain_bn(cc):
                s = st[cc]
                mv = spool.tile([128, 2], f32, tag="mv", name=f"mv{cc}")
                nc.vector.bn_aggr(mv[:], s["stats6"][:].rearrange(
                    "p (h s) -> p h s", s=6))
                _finish_chain(cc, mv[:, 0:1], mv[:, 1:2])

            def emit_chain_acc(cc):
                s = st[cc]
                acc = s["acc"]
                sm = spool.tile([128, 2], f32, tag="sm", name=f"sm{cc}")
                nc.vector.tensor_reduce(sm[:, 0:1], acc[:, 0:4],
                                        mybir.AxisListType.X, ADD)
                nc.vector.tensor_reduce(sm[:, 1:2], acc[:, 4:8],
                                        mybir.AxisListType.X, ADD)
                mean = spool.tile([128, 1], f32, tag="mean", name=f"me{cc}")
                nc.vector.tensor_scalar_mul(mean[:], sm[:, 0:1], 1.0 / HW)
                ex2 = spool.tile([128, 1], f32, tag="ex2", name=f"e2{cc}")
                nc.vector.tensor_scalar_mul(ex2[:], sm[:, 1:2], 1.0 / HW)
                m2 = spool.tile([128, 1], f32, tag="m2", name=f"m2{cc}")
                nc.vector.tensor_mul(m2[:], mean[:], mean[:])
                var = spool.tile([128, 1], f32, tag="var", name=f"va{cc}")
                nc.vector.tensor_sub(var[:], ex2[:], m2[:])
                _finish_chain(cc, mean[:], var[:])

            def _finish_chain(cc, mean_ap, var_ap):
                s = st[cc]
                stdv = spool.tile([128, 1], f32, tag="stdv", name=f"sd{cc}")
                nc.scalar.activation(stdv[:], var_ap,
                                     mybir.ActivationFunctionType.Sqrt,
                                     scale=float(HW) / float(HW - 1))
                stde = spool.tile([128, 1], f32, tag="stde", name=f"se{cc}")
                nc.vector.tensor_scalar_add(stde[:], stdv[:], EPS)
                rstd = spool.tile([128, 1], f32, tag="rstd", name=f"rs{cc}")
                nc.vector.reciprocal(rstd[:], stde[:])
                nmr = spool.tile([128, 1], f32, tag="nmr", name=f"nm{cc}")
                nc.vector.scalar_tensor_tensor(nmr[:], mean_ap, -1.0, rstd[:],
                                               op0=MULT, op1=MULT)
                s["rstd"] = rstd
                s["nmr"] = nmr

            def emit_norm(cc, engines=(None,)):
                # 8 row-band pieces; piece q covers rows [4q,4q+4) of each band
                s = st[cc]
                full = s["xt"][:, W:W + HW].rearrange("p (b r w) -> p b r w",
                                                      b=4, r=32)
                for q in range(8):
                    eng = engines[q % len(engines)]
                    tgt = full[:, :, 4 * q:4 * q + 4, :]
                    if eng is nc.scalar:
                        nc.scalar.activation(tgt, tgt, IDENT,
                                             bias=s["nmr"][:, 0:1],
                                             scale=s["rstd"][:, 0:1])
                    else:
                        eng.tensor_scalar(tgt, tgt, s["rstd"][:, 0:1],
                                          s["nmr"][:, 0:1], op0=MULT, op1=ADD)

            def emit_quad_mms(cc, q):
                xt = st[cc]["xt"]
                pb = [pspool.tile([128, 512], f32, tag="pb",
                                  name=f"pb{cc}_{q}_{r}") for r in range(4)]
                for ti, (dh, dwi) in enumerate(TAPS):
                    start, stop = (ti == 0), (ti == NTAP - 1)
                    tapi = dh * 3 + dwi
                    for r in range(4):
                        lhsT = w_sb[32 * r:32 * r + 32,
                                    (cc * NTAP + tapi) * 32:
                                    (cc * NTAP + tapi) * 32 + 32]
                        for c in range(4):
                            t = 8 * c + q
                            base = (4 * t + dh) * W
                            outp = pb[r][32 * c:32 * c + 32, :]
                            tp = (32 * r, 32 * c)
                            if dwi == 1:
                                nc.tensor.matmul(
                                    outp, lhsT,
                                    xt[32 * r:32 * r + 32, base:base + 512],
                                    start=start, stop=stop, tile_position=tp)
                            else:
                                o3 = outp.rearrange("p (h w) -> p h w", w=W)
                                r3 = xt[32 * r:32 * r + 32,
                                        base:base + 512].rearrange(
                                            "p (h w) -> p h w", w=W)
                                if dwi == 0:   # dw=-1
                                    nc.tensor.matmul(
                                        o3[:, :, 1:W], lhsT, r3[:, :, 0:W - 1],
                                        start=start, stop=stop,
                                        skip_group_check=True, tile_position=tp)
                                else:          # dw=+1
                                    nc.tensor.matmul(
                                        o3[:, :, 0:W - 1], lhsT, r3[:, :, 1:W],
                                        start=start, stop=stop,
                                        skip_group_check=True, tile_position=tp)
                return pb

            def emit_evac(cc, q, pb, om):
                for r in range(4):
                    bias_ap = bias_sb[:, cc * 4 + r: cc * 4 + r + 1]
                    dst = om[:, r * 4096 + q * 512: r * 4096 + q * 512 + 512]
                    nc.vector.tensor_scalar_add(dst, pb[r][:, :], bias_ap)

            def emit_out(cc, om, half):
                for c in range(4):
                    nc.sync.dma_start(
                        out_v[cc, c, half],
                        om[32 * c:32 * c + 32, :].rearrange(
                            "p (r h e) -> p r h e", h=2, e=2048)[:, :, half, :])

            # ---- prologue: chunk 0 (and chunk 1 load) up front
            emit_load(0)
            emit_load(1)
            for blk in range(4):
                emit_stats_block(0, blk)
            emit_chain_bn(0)
            emit_norm(0, engines=(nc.gpsimd, nc.vector, nc.scalar))

            for cc in range(NCH):
                om = opool.tile([128, 4 * 4096], bf16, tag="om", name=f"om{cc}")
                for q in range(8):
                    pb = emit_quad_mms(cc, q)
                    emit_evac(cc, q, pb, om)
                    if cc + 1 < NCH:
                        if q == 0 and cc + 2 < NCH:
                            emit_load(cc + 2)
                        if 1 <= q <= 4:
                            emit_stats_act(cc + 1, q - 1)
                        if q == 5:
                            emit_chain_acc(cc + 1)
                        if q == 6:
                            emit_norm(cc + 1, engines=(nc.gpsimd,))
                    if q == 3:
                        emit_out(cc, om, 0)
                emit_out(cc, om, 1)
    nc.compile()
    return nc


def _pack_inputs(x, dw, pw, biases):
    """Host-side: fuse pw o dw, scatter into block-diag 32x32 lhsT tiles."""
    G = 128
    dwr = dw.reshape(B, G, 4, 4, 3, 3)          # [b, g, m, i, kh, kw]
    pwr = pw.reshape(B, G, 4, 4)                # [b, g, j, m]
    eff = np.einsum('bgjm,bgmikl->bgjikl', pwr, dwr)  # [b, g, j, i, kh, kw]
    w_host = np.zeros((B, 128, NCH * NTAP * 32), dtype=np.float32)
    wv = w_host.reshape(B, 4, 8, 4, NCH, NTAP, 8, 4)  # [b, r, gl_k, i, cc, tap, gl_m, j]
    er = eff.reshape(B, NCH, 4, 8, 4, 4, NTAP)        # [b, cc, r, gl, j, i, tap]
    for gl in range(8):
        e = er[:, :, :, gl]                     # [b, cc, r, j, i, tap]
        wv[:, :, gl, :, :, :, gl, :] = e.transpose(0, 2, 4, 1, 5, 3)
    bias_host = np.zeros((B, 128, 16), dtype=np.float32)
    bfull = biases.reshape(B, C)
    p = np.arange(128)
    for cc in range(NCH):
        for r in range(4):
            bias_host[:, :, cc * 4 + r] = bfull[:, cc * 128 + 32 * r + (p % 32)]
    return w_host.astype(ml_dtypes.bfloat16), bias_host


def kernel(x, dw_kernels, pw_kernels, biases):
    from concourse.bass_utils import run_bass_kernel_spmd

    x = np.ascontiguousarray(np.asarray(x, dtype=np.float32))
    dw = np.asarray(dw_kernels, dtype=np.float32)
    pw = np.asarray(pw_kernels, dtype=np.float32)
    bs = np.asarray(biases, dtype=np.float32)

    if "nc" not in _CACHE:
        _CACHE["nc"] = _build_program()
    nc = _CACHE["nc"]

    w_host, bias_host = _pack_inputs(x, dw, pw, bs)
    in_maps = [{"x": x[i].reshape(C, HW),
                "w": w_host[i],
                "bias": bias_host[i]} for i in range(B)]
    res = run_bass_kernel_spmd(nc, in_maps, core_ids=list(range(B)),
                               trace=bool(int(os.environ.get("KTRACE", "0"))))
    _CACHE["last_result"] = res
    out = np.stack([res.results[i]["out"].astype(np.float32).reshape(C, H, W)
                    for i in range(B)])
    return out
